# revision 17
# baseline (speedup 1.0000x reference)
"""Expert-parallel MoE grouped-experts kernel for 8 trn2 NeuronCores.

Contract: kernel(**inputs) takes FULL unsharded inputs, returns FULL output.

Strategy (expert-parallel, load-balanced):
  - Host: sort token-expert assignments by expert. Sort experts by size and
    deal them round-robin into 4 "slots" x 8 cores (slot i holds the experts
    ranked [8i, 8i+8)), so the per-slot padded size S_i = max over its 8
    experts is tight. Every core runs the same program: 4 expert blocks of
    S_0..S_3 rows (128-padded).
  - Device (SPMD x8), all bf16 matmuls with fp32 PSUM accumulation:
      g = x @ gwT, u = x @ uwT, hmid = silu(g)*u (bf16), o = hmid @ dwT.
    Activations arrive transposed [H, CT]; weights streamed per i-tile
    (gate/up) or slot-resident (down). Activation chunks double-buffered.
  - Host: scale by routing weights, scatter-add back to token order.
"""
import sys

if "/opt/trn_rl_repo" not in sys.path:
    sys.path.insert(0, "/opt/trn_rl_repo")

import math
import numpy as np
import ml_dtypes

B, S, H, I, E, K = 4, 4096, 2048, 1024, 32, 4
N = B * S
NCORES = 8
NSLOT = E // NCORES  # 4 expert slots per core
HC = H // 128        # 16 h-chunks (contraction for gate/up)
IC = I // 128        # 8 i-chunks (contraction for down)
IT = I // 128        # 8 i-tiles of 128 (gate/up output tiles)
HS = H // 512        # 4 h output slices of 512 (down output)

CHUNK_MAX = 1152     # rows per resident activation chunk (multiple of 128)
FIRST_CHUNK = 640    # smaller first chunk so the first matmul starts early

_LAST_RESULTS = None  # BassKernelResults of the most recent run (for test.py)


def _split32(R: int, maxlen: int):
    """Split R (multiple of 32) into nearly equal pieces <= maxlen, all
    multiples of 32."""
    u = R // 32
    n = max(1, math.ceil(u / (maxlen // 32)))
    base, rem = divmod(u, n)
    return [32 * (base + (1 if i < rem else 0)) for i in range(n)]


def _chunks(R: int, first_small: bool = False):
    """Split R rows (multiple of 32) into chunks <= CHUNK_MAX. first_small
    carves a small leading chunk so the kernel's first matmuls only wait on a
    small activation DMA."""
    pre = []
    r = 0
    if first_small and R > FIRST_CHUNK:
        pre = [(0, FIRST_CHUNK)]
        r = FIRST_CHUNK
        R = R - FIRST_CHUNK
    out = []
    for cl in _split32(R, CHUNK_MAX):
        out.append((r, cl))
        r += cl
    return pre + out


def _slices(CR: int):
    """Split chunk cols into equal slices <= 512 (PSUM bank limit)."""
    out = []
    r = 0
    for sl in _split32(CR, 512):
        out.append((r, sl))
        r += sl
    return out


def _build(S_tiles, chunk_lists):
    """S_tiles: per-slot padded sizes in 128-row tiles (same on all cores).
    chunk_lists[si]: list of (offset, rows) chunks for slot si."""
    import concourse.tile as tile
    import concourse.mybir as mybir
    from concourse import bacc

    bf16 = mybir.dt.bfloat16
    f32 = mybir.dt.float32

    CT = 128 * sum(S_tiles)
    NCH = sum(len(c) for c in chunk_lists)

    nc = bacc.Bacc("TRN2", target_bir_lowering=False, debug=False)

    # xsP[k]: chunk k's activations, [HC, 128, CHUNK_MAX] (h-chunk, h%128, row)
    xsP = nc.dram_tensor("xsP", [NCH, HC, 128, CHUNK_MAX], bf16, kind="ExternalInput")
    # gwP[s, it, p, hc, il] = gate[e_s, it*128+il, hc*128+p]
    gwP = nc.dram_tensor("gwP", [NSLOT, IT, 128, HC, 128], bf16, kind="ExternalInput")
    uwP = nc.dram_tensor("uwP", [NSLOT, IT, 128, HC, 128], bf16, kind="ExternalInput")
    # dwP[s, hs, p, ic, h'] = down[e_s, hs*512+h', ic*128+p]
    dwP = nc.dram_tensor("dwP", [NSLOT, HS, 128, IC, 512], bf16, kind="ExternalInput")
    o = nc.dram_tensor("o", [CT, H], bf16, kind="ExternalOutput")

    with tile.TileContext(nc) as tc:
        with (
            tc.tile_pool(name="xs", bufs=2) as xsp,
            tc.tile_pool(name="hm", bufs=2) as hmp,
            tc.tile_pool(name="wg", bufs=3) as wg,
            tc.tile_pool(name="wu", bufs=3) as wu,
            tc.tile_pool(name="wd", bufs=6) as wd,
            tc.tile_pool(name="sg", bufs=2) as sgp,
            tc.tile_pool(name="ost", bufs=4) as ostp,
            tc.tile_pool(name="psum", bufs=8, space="PSUM") as psp,
        ):
            base = 0
            kch = 0
            for si in range(NSLOT):
                R = 128 * S_tiles[si]

                for ci, (c0, CR) in enumerate(chunk_lists[si]):
                    cb = base + c0
                    xst_full = xsp.tile([128, HC, CHUNK_MAX], bf16, tag="xs")
                    xst = xst_full[:, :, :CR]
                    if si == 0 and ci == 0:
                        # split at the slice boundary: the first matmul group
                        # only waits on the first half of the chunk
                        h0 = _slices(CR)[0][1]
                        nc.sync.dma_start(
                            xst[:, :, :h0],
                            xsP.ap()[kch][:, :, :h0].rearrange("c p r -> p c r"),
                        )
                        nc.sync.dma_start(
                            xst[:, :, h0:CR],
                            xsP.ap()[kch][:, :, h0:CR].rearrange("c p r -> p c r"),
                        )
                    else:
                        nc.sync.dma_start(
                            xst[:],
                            xsP.ap()[kch][:, :, :CR].rearrange("c p r -> p c r"),
                        )
                    kch += 1
                    if ci == 0:
                        # down weights resident for the whole slot (4 x 1MB);
                        # on the sync queue so they sit behind the activation
                        # chunk, not in front of the gate/up weight stream
                        dwS = []
                        for hs in range(HS):
                            dwt = wd.tile([128, IC, 512], bf16, tag="dw")
                            nc.sync.dma_start(dwt[:], dwP.ap()[si, hs])
                            dwS.append(dwt)
                    hm_full = hmp.tile([128, IC, CHUNK_MAX], bf16, tag="hm")
                    hm = hm_full[:, :, :CR]

                    sl_list = _slices(CR)
                    for it in range(IT):
                        gw = wg.tile([128, HC, 128], bf16, tag="gw")
                        nc.scalar.dma_start(gw[:], gwP.ap()[si, it])
                        uw = wu.tile([128, HC, 128], bf16, tag="uw")
                        nc.scalar.dma_start(uw[:], uwP.ap()[si, it])
                        for r0, rl in sl_list:
                            pg_full = psp.tile([128, 512], f32, tag="ps")
                            pu_full = psp.tile([128, 512], f32, tag="ps")
                            pg = pg_full[:, :rl]
                            pu = pu_full[:, :rl]
                            for hc in range(HC):
                                nc.tensor.matmul(
                                    pg[:], gw[:, hc, :], xst[:, hc, r0 : r0 + rl],
                                    start=(hc == 0), stop=(hc == HC - 1),
                                )
                            for hc in range(HC):
                                nc.tensor.matmul(
                                    pu[:], uw[:, hc, :], xst[:, hc, r0 : r0 + rl],
                                    start=(hc == 0), stop=(hc == HC - 1),
                                )
                            sg_full = sgp.tile([128, 512], f32, tag="sg")
                            sg = sg_full[:, :rl]
                            nc.scalar.activation(
                                sg[:], pg[:], mybir.ActivationFunctionType.Silu
                            )
                            nc.vector.tensor_mul(hm[:, it, r0 : r0 + rl], sg[:], pu[:])

                    # last chunk may be clipped below a 128 multiple: the last
                    # row-tile then reads some unwritten hm cols (within the
                    # allocation) and produces garbage padding rows in o that
                    # the host never reads back
                    for rt in range(math.ceil(CR / 128)):
                        ot = ostp.tile([128, H], bf16, tag="o")
                        for hs in range(HS):
                            po = psp.tile([128, 512], f32, tag="ps")
                            for ic in range(IC):
                                nc.tensor.matmul(
                                    po[:], hm_full[:, ic, rt * 128 : (rt + 1) * 128],
                                    dwS[hs][:, ic, :],
                                    start=(ic == 0), stop=(ic == IC - 1),
                                )
                            nc.vector.tensor_copy(ot[:, hs * 512 : (hs + 1) * 512], po[:])
                        nc.sync.dma_start(
                            o.ap()[cb + rt * 128 : cb + (rt + 1) * 128, :], ot[:]
                        )
                base += R
    nc.compile()
    return nc


def kernel(hidden_states, gate_weight, up_weight, down_weight, topk_idx, topk_weight):
    global _LAST_RESULTS
    from concourse.bass_utils import run_bass_kernel_spmd

    bf16 = ml_dtypes.bfloat16

    x = np.ascontiguousarray(hidden_states, dtype=np.float32).reshape(N, H)
    flat_expert = np.asarray(topk_idx).reshape(-1).astype(np.int64)
    flat_weight = np.asarray(topk_weight).reshape(-1).astype(np.float32)

    perm = np.argsort(flat_expert, kind="stable")
    tok_sorted = np.repeat(np.arange(N), K)[perm]
    sizes = np.bincount(flat_expert, minlength=E)
    offs = np.concatenate([[0], np.cumsum(sizes)])

    # slot assignment: sort experts by size desc; slot i <- ranks [8i, 8i+8)
    order = np.argsort(-sizes, kind="stable")
    expert_of = order.reshape(NSLOT, NCORES)  # [slot, core] -> expert id
    S_tiles = [
        max(1, int(math.ceil(sizes[expert_of[i]].max() / 128))) for i in range(NSLOT)
    ]
    CT = 128 * sum(S_tiles)
    slot_base = np.concatenate([[0], np.cumsum([128 * s for s in S_tiles])])

    gw_all = np.asarray(gate_weight, dtype=np.float32)
    uw_all = np.asarray(up_weight, dtype=np.float32)
    dw_all = np.asarray(down_weight, dtype=np.float32)

    def pack_gu(w):  # w: (NSLOT, I, H) -> [NSLOT, IT, 128p, HC, 128il]
        w5 = w.reshape(NSLOT, IT, 128, HC, 128)  # (s, it, il, hc, p)
        return np.ascontiguousarray(w5.transpose(0, 1, 4, 3, 2)).astype(bf16)

    def pack_dw(w):  # w: (NSLOT, H, I) -> [NSLOT, HS, 128p, IC, 512h']
        w5 = w.reshape(NSLOT, HS, 512, IC, 128)  # (s, hs, h', ic, p)
        return np.ascontiguousarray(w5.transpose(0, 1, 4, 3, 2)).astype(bf16)

    # gate/up only needs to cover the real rows (ceil32 of slot max); the o
    # region stays 128*S_tiles so down's final partial row-tile has room
    R32 = [int(math.ceil(sizes[expert_of[i]].max() / 32) * 32) for i in range(NSLOT)]
    chunk_lists = [_chunks(R32[si], first_small=(si == 0)) for si in range(NSLOT)]
    NCH = sum(len(c) for c in chunk_lists)

    in_maps = []
    for m in range(NCORES):
        ex = expert_of[:, m]  # 4 expert ids for this core
        xsT_m = np.zeros((H, CT), dtype=bf16)
        for si in range(NSLOT):
            e = ex[si]
            ids = tok_sorted[offs[e] : offs[e + 1]]
            if len(ids):
                xsT_m[:, slot_base[si] : slot_base[si] + len(ids)] = (
                    x[ids].astype(bf16).T
                )
        # repack per-chunk contiguous: xsP[k, hc, p, r]
        xsP_m = np.zeros((NCH, HC, 128, CHUNK_MAX), dtype=bf16)
        k = 0
        for si in range(NSLOT):
            for c0, CR in chunk_lists[si]:
                cb = slot_base[si] + c0
                xsP_m[k, :, :, :CR] = xsT_m[:, cb : cb + CR].reshape(HC, 128, CR)
                k += 1
        in_maps.append(
            {
                "xsP": xsP_m,
                "gwP": pack_gu(gw_all[ex]),
                "uwP": pack_gu(uw_all[ex]),
                "dwP": pack_dw(dw_all[ex]),
            }
        )

    nc = _build(S_tiles, chunk_lists)
    try:
        res = run_bass_kernel_spmd(nc, in_maps, core_ids=list(range(NCORES)))
    except Exception:
        # transient device errors (e.g. NRT_EXEC_UNIT_UNRECOVERABLE from a
        # wedged core) usually clear on re-execute
        res = run_bass_kernel_spmd(nc, in_maps, core_ids=list(range(NCORES)))
    _LAST_RESULTS = res

    # combine: weighted scatter-add back to token order
    o_sorted = np.empty((N * K, H), dtype=np.float32)
    for m in range(NCORES):
        om = res.results[m]["o"]
        for si in range(NSLOT):
            e = expert_of[si, m]
            n_e = offs[e + 1] - offs[e]
            o_sorted[offs[e] : offs[e + 1]] = om[
                slot_base[si] : slot_base[si] + n_e
            ].astype(np.float32)
    o_sorted *= flat_weight[perm][:, None]
    o_orig = np.empty_like(o_sorted)
    o_orig[perm] = o_sorted
    y = o_orig.reshape(N, K, H).sum(axis=1)
    return y.reshape(B, S, H).astype(np.float32)


# revision 18
# speedup vs baseline: 1.0023x; 1.0023x over previous
"""Expert-parallel MoE grouped-experts kernel for 8 trn2 NeuronCores.

Contract: kernel(**inputs) takes FULL unsharded inputs, returns FULL output.

Strategy (expert-parallel, load-balanced):
  - Host: sort token-expert assignments by expert. Sort experts by size and
    deal them round-robin into 4 "slots" x 8 cores (slot i holds the experts
    ranked [8i, 8i+8)), so the per-slot padded size S_i = max over its 8
    experts is tight. Every core runs the same program: 4 expert blocks of
    S_0..S_3 rows (128-padded).
  - Device (SPMD x8), all bf16 matmuls with fp32 PSUM accumulation:
      g = x @ gwT, u = x @ uwT, hmid = silu(g)*u (bf16), o = hmid @ dwT.
    Activations arrive transposed [H, CT]; weights streamed per i-tile
    (gate/up) or slot-resident (down). Activation chunks double-buffered.
  - Host: scale by routing weights, scatter-add back to token order.
"""
import sys

if "/opt/trn_rl_repo" not in sys.path:
    sys.path.insert(0, "/opt/trn_rl_repo")

import math
import numpy as np
import ml_dtypes

B, S, H, I, E, K = 4, 4096, 2048, 1024, 32, 4
N = B * S
NCORES = 8
NSLOT = E // NCORES  # 4 expert slots per core
HC = H // 128        # 16 h-chunks (contraction for gate/up)
IC = I // 128        # 8 i-chunks (contraction for down)
IT = I // 128        # 8 i-tiles of 128 (gate/up output tiles)
HS = H // 512        # 4 h output slices of 512 (down output)

CHUNK_MAX = 1152     # rows per resident activation chunk (multiple of 128)
FIRST_CHUNK = 640    # smaller first chunk so the first matmul starts early

_LAST_RESULTS = None  # BassKernelResults of the most recent run (for test.py)


def _split32(R: int, maxlen: int):
    """Split R (multiple of 32) into nearly equal pieces <= maxlen, all
    multiples of 32."""
    u = R // 32
    n = max(1, math.ceil(u / (maxlen // 32)))
    base, rem = divmod(u, n)
    return [32 * (base + (1 if i < rem else 0)) for i in range(n)]


def _chunks(R: int, first_small: bool = False):
    """Split R rows (multiple of 32) into chunks <= CHUNK_MAX. first_small
    carves a small leading chunk so the kernel's first matmuls only wait on a
    small activation DMA."""
    pre = []
    r = 0
    if first_small and R > FIRST_CHUNK:
        pre = [(0, FIRST_CHUNK)]
        r = FIRST_CHUNK
        R = R - FIRST_CHUNK
    out = []
    for cl in _split32(R, CHUNK_MAX):
        out.append((r, cl))
        r += cl
    return pre + out


def _slices(CR: int):
    """Split chunk cols into equal slices <= 512 (PSUM bank limit)."""
    out = []
    r = 0
    for sl in _split32(CR, 512):
        out.append((r, sl))
        r += sl
    return out


def _build(S_tiles, chunk_lists):
    """S_tiles: per-slot padded sizes in 128-row tiles (same on all cores).
    chunk_lists[si]: list of (offset, rows) chunks for slot si."""
    import concourse.tile as tile
    import concourse.mybir as mybir
    from concourse import bacc

    bf16 = mybir.dt.bfloat16
    f32 = mybir.dt.float32

    CT = 128 * sum(S_tiles)
    NCH = sum(len(c) for c in chunk_lists)

    nc = bacc.Bacc("TRN2", target_bir_lowering=False, debug=False)

    # xsP[k]: chunk k's activations, [HC, 128, CHUNK_MAX] (h-chunk, h%128, row)
    xsP = nc.dram_tensor("xsP", [NCH, HC, 128, CHUNK_MAX], bf16, kind="ExternalInput")
    # gwP[s, it, p, hc, il] = gate[e_s, it*128+il, hc*128+p]
    gwP = nc.dram_tensor("gwP", [NSLOT, IT, 128, HC, 128], bf16, kind="ExternalInput")
    uwP = nc.dram_tensor("uwP", [NSLOT, IT, 128, HC, 128], bf16, kind="ExternalInput")
    # dwP[s, hs, p, ic, h'] = down[e_s, hs*512+h', ic*128+p]
    dwP = nc.dram_tensor("dwP", [NSLOT, HS, 128, IC, 512], bf16, kind="ExternalInput")
    o = nc.dram_tensor("o", [CT, H], bf16, kind="ExternalOutput")

    with tile.TileContext(nc) as tc:
        with (
            tc.tile_pool(name="xs", bufs=2) as xsp,
            tc.tile_pool(name="hm", bufs=2) as hmp,
            tc.tile_pool(name="wg", bufs=2) as wg,
            tc.tile_pool(name="wu", bufs=2) as wu,
            tc.tile_pool(name="wd", bufs=6) as wd,
            tc.tile_pool(name="sg", bufs=2) as sgp,
            tc.tile_pool(name="ost", bufs=3) as ostp,
            tc.tile_pool(name="psum", bufs=8, space="PSUM") as psp,
        ):
            base = 0
            kch = 0
            for si in range(NSLOT):
                R = 128 * S_tiles[si]

                for ci, (c0, CR) in enumerate(chunk_lists[si]):
                    cb = base + c0
                    xst_full = xsp.tile([128, HC, CHUNK_MAX], bf16, tag="xs")
                    xst = xst_full[:, :, :CR]
                    if si == 0 and ci == 0:
                        # split at the slice boundary: the first matmul group
                        # only waits on the first half of the chunk
                        h0 = _slices(CR)[0][1]
                        nc.sync.dma_start(
                            xst[:, :, :h0],
                            xsP.ap()[kch][:, :, :h0].rearrange("c p r -> p c r"),
                        )
                        nc.sync.dma_start(
                            xst[:, :, h0:CR],
                            xsP.ap()[kch][:, :, h0:CR].rearrange("c p r -> p c r"),
                        )
                    else:
                        nc.sync.dma_start(
                            xst[:],
                            xsP.ap()[kch][:, :, :CR].rearrange("c p r -> p c r"),
                        )
                    kch += 1
                    if ci == 0:
                        # down weights resident for the whole slot (4 x 1MB);
                        # on the sync queue so they sit behind the activation
                        # chunk, not in front of the gate/up weight stream
                        dwS = []
                        for hs in range(HS):
                            dwt = wd.tile([128, IC, 512], bf16, tag="dw")
                            nc.sync.dma_start(dwt[:], dwP.ap()[si, hs])
                            dwS.append(dwt)
                    hm_full = hmp.tile([128, IC, CHUNK_MAX], bf16, tag="hm")
                    hm = hm_full[:, :, :CR]

                    sl_list = _slices(CR)
                    for it in range(IT):
                        gw = wg.tile([128, HC, 128], bf16, tag="gw")
                        nc.scalar.dma_start(gw[:], gwP.ap()[si, it])
                        uw = wu.tile([128, HC, 128], bf16, tag="uw")
                        nc.scalar.dma_start(uw[:], uwP.ap()[si, it])
                        for r0, rl in sl_list:
                            pg_full = psp.tile([128, 512], f32, tag="ps")
                            pu_full = psp.tile([128, 512], f32, tag="ps")
                            pg = pg_full[:, :rl]
                            pu = pu_full[:, :rl]
                            for hc in range(HC):
                                nc.tensor.matmul(
                                    pg[:], gw[:, hc, :], xst[:, hc, r0 : r0 + rl],
                                    start=(hc == 0), stop=(hc == HC - 1),
                                )
                            for hc in range(HC):
                                nc.tensor.matmul(
                                    pu[:], uw[:, hc, :], xst[:, hc, r0 : r0 + rl],
                                    start=(hc == 0), stop=(hc == HC - 1),
                                )
                            sg_full = sgp.tile([128, 512], f32, tag="sg")
                            sg = sg_full[:, :rl]
                            nc.scalar.activation(
                                sg[:], pg[:], mybir.ActivationFunctionType.Silu
                            )
                            nc.vector.tensor_mul(hm[:, it, r0 : r0 + rl], sg[:], pu[:])

                    # last chunk may be clipped below a 128 multiple: the last
                    # row-tile then reads some unwritten hm cols (within the
                    # allocation) and produces garbage padding rows in o that
                    # the host never reads back
                    for rt in range(math.ceil(CR / 128)):
                        ot = ostp.tile([128, H], bf16, tag="o")
                        for hs in range(HS):
                            po = psp.tile([128, 512], f32, tag="ps")
                            for ic in range(IC):
                                nc.tensor.matmul(
                                    po[:], hm_full[:, ic, rt * 128 : (rt + 1) * 128],
                                    dwS[hs][:, ic, :],
                                    start=(ic == 0), stop=(ic == IC - 1),
                                )
                            nc.vector.tensor_copy(ot[:, hs * 512 : (hs + 1) * 512], po[:])
                        nc.sync.dma_start(
                            o.ap()[cb + rt * 128 : cb + (rt + 1) * 128, :], ot[:]
                        )
                base += R
    nc.compile()
    return nc


def kernel(hidden_states, gate_weight, up_weight, down_weight, topk_idx, topk_weight):
    global _LAST_RESULTS
    from concourse.bass_utils import run_bass_kernel_spmd

    bf16 = ml_dtypes.bfloat16

    x = np.ascontiguousarray(hidden_states, dtype=np.float32).reshape(N, H)
    flat_expert = np.asarray(topk_idx).reshape(-1).astype(np.int64)
    flat_weight = np.asarray(topk_weight).reshape(-1).astype(np.float32)

    perm = np.argsort(flat_expert, kind="stable")
    tok_sorted = np.repeat(np.arange(N), K)[perm]
    sizes = np.bincount(flat_expert, minlength=E)
    offs = np.concatenate([[0], np.cumsum(sizes)])

    # slot assignment: sort experts by size desc; slot i <- ranks [8i, 8i+8)
    order = np.argsort(-sizes, kind="stable")
    expert_of = order.reshape(NSLOT, NCORES)  # [slot, core] -> expert id
    S_tiles = [
        max(1, int(math.ceil(sizes[expert_of[i]].max() / 128))) for i in range(NSLOT)
    ]
    CT = 128 * sum(S_tiles)
    slot_base = np.concatenate([[0], np.cumsum([128 * s for s in S_tiles])])

    gw_all = np.asarray(gate_weight, dtype=np.float32)
    uw_all = np.asarray(up_weight, dtype=np.float32)
    dw_all = np.asarray(down_weight, dtype=np.float32)

    def pack_gu(w):  # w: (NSLOT, I, H) -> [NSLOT, IT, 128p, HC, 128il]
        w5 = w.reshape(NSLOT, IT, 128, HC, 128)  # (s, it, il, hc, p)
        return np.ascontiguousarray(w5.transpose(0, 1, 4, 3, 2)).astype(bf16)

    def pack_dw(w):  # w: (NSLOT, H, I) -> [NSLOT, HS, 128p, IC, 512h']
        w5 = w.reshape(NSLOT, HS, 512, IC, 128)  # (s, hs, h', ic, p)
        return np.ascontiguousarray(w5.transpose(0, 1, 4, 3, 2)).astype(bf16)

    # gate/up only needs to cover the real rows (ceil32 of slot max); the o
    # region stays 128*S_tiles so down's final partial row-tile has room
    R32 = [int(math.ceil(sizes[expert_of[i]].max() / 32) * 32) for i in range(NSLOT)]
    chunk_lists = [_chunks(R32[si], first_small=(si == 0)) for si in range(NSLOT)]
    NCH = sum(len(c) for c in chunk_lists)

    in_maps = []
    for m in range(NCORES):
        ex = expert_of[:, m]  # 4 expert ids for this core
        xsT_m = np.zeros((H, CT), dtype=bf16)
        for si in range(NSLOT):
            e = ex[si]
            ids = tok_sorted[offs[e] : offs[e + 1]]
            if len(ids):
                xsT_m[:, slot_base[si] : slot_base[si] + len(ids)] = (
                    x[ids].astype(bf16).T
                )
        # repack per-chunk contiguous: xsP[k, hc, p, r]
        xsP_m = np.zeros((NCH, HC, 128, CHUNK_MAX), dtype=bf16)
        k = 0
        for si in range(NSLOT):
            for c0, CR in chunk_lists[si]:
                cb = slot_base[si] + c0
                xsP_m[k, :, :, :CR] = xsT_m[:, cb : cb + CR].reshape(HC, 128, CR)
                k += 1
        in_maps.append(
            {
                "xsP": xsP_m,
                "gwP": pack_gu(gw_all[ex]),
                "uwP": pack_gu(uw_all[ex]),
                "dwP": pack_dw(dw_all[ex]),
            }
        )

    nc = _build(S_tiles, chunk_lists)
    try:
        res = run_bass_kernel_spmd(nc, in_maps, core_ids=list(range(NCORES)))
    except Exception:
        # transient device errors (e.g. NRT_EXEC_UNIT_UNRECOVERABLE from a
        # wedged core) usually clear on re-execute
        res = run_bass_kernel_spmd(nc, in_maps, core_ids=list(range(NCORES)))
    _LAST_RESULTS = res

    # combine: weighted scatter-add back to token order
    o_sorted = np.empty((N * K, H), dtype=np.float32)
    for m in range(NCORES):
        om = res.results[m]["o"]
        for si in range(NSLOT):
            e = expert_of[si, m]
            n_e = offs[e + 1] - offs[e]
            o_sorted[offs[e] : offs[e + 1]] = om[
                slot_base[si] : slot_base[si] + n_e
            ].astype(np.float32)
    o_sorted *= flat_weight[perm][:, None]
    o_orig = np.empty_like(o_sorted)
    o_orig[perm] = o_sorted
    y = o_orig.reshape(N, K, H).sum(axis=1)
    return y.reshape(B, S, H).astype(np.float32)
